# revision 4
# baseline (speedup 1.0000x reference)
"""Paged KV-cache gather + dequant kernel for 8 Trainium2 NeuronCores.

Problem: out[0] = zeros; out[1+i] = kv_cache[block_tables.flat[i]] * scale
(k_scale for the K half, v_scale for the V half), zeroed where the table
entry is <= 0.  Shapes: kv_cache [4096, 2, 8, 16, 128] fp16,
block_tables [32, 128] int, out [4097, 2, 8, 16, 128] fp16.

Sharding: batch across the 8 cores (4 sequences = 512 entries per core);
kv_cache replicated.  Per core the kernel views kv_cache as 4096 blocks
of 2x16384 fp16 (64 KB: the K and V halves of one block, contiguous) and
pipelines 4 chunks of 128 blocks through 3 SBUF buffers:
  1. loads block_tables partition-major, builds int32 block indices
     (clamped >= 0) and per-entry scales ((bt > 0) * {k,v}_scale) on DVE,
  2. gathers 128 blocks per chunk with an indirect (DynamicAP) DMA on the
     gpsimd dynamic queue -- mainline SWDGE, no ucode library load,
  3. dequantizes in SBUF: DVE scales the K half, ACT the V half
     (per-partition scalar multiply),
  4. stores chunks to the output shard with HWDGE DMAs.
Invalid entries gather block 0 and are zeroed by the scale; output block 0
is never written (ExternalOutput buffers are zero-initialized).
"""

import sys

if "/opt/trn_rl_repo" not in sys.path:
    sys.path.insert(0, "/opt/trn_rl_repo")

from contextlib import ExitStack

import numpy as np

import concourse.bacc as bacc
import concourse.bass as bass
import concourse.mybir as mybir
from concourse import bass_utils
from concourse._compat import get_trn_type

N_CORES = 8
NUM_BLOCKS, NUM_KV_HEADS, HEAD_DIM, BLOCK_SIZE = 4096, 8, 128, 16
BATCH, MAX_BLOCKS_PER_SEQ = 32, 128

ROW = NUM_KV_HEADS * BLOCK_SIZE * HEAD_DIM            # 16384 fp16 = one K/V half
E_PER_CORE = (BATCH // N_CORES) * MAX_BLOCKS_PER_SEQ  # 512 entries per core

_NC_CACHE = None


def build_nc(
    n_reps: int = 1,
    chunk: int = 128,
    n_buf: int = 3,
    split_store: bool = False,
) -> bass.Bass:
    # Bacc handles BIR lowering; n_reps > 1 unrolls the main loop for
    # benchmarking (idempotent: each rep re-gathers before scaling).
    nc = bacc.Bacc(get_trn_type() or "TRN2")

    n_chunk = E_PER_CORE // chunk   # chunks per rep
    cols = E_PER_CORE // 128        # idx columns (entry c*128+p -> [p, c])

    kv = nc.dram_tensor(
        "kv", [NUM_BLOCKS, 2 * ROW], mybir.dt.float16, kind="ExternalInput"
    )
    bt = nc.dram_tensor("bt", [E_PER_CORE], mybir.dt.int32, kind="ExternalInput")
    scales = nc.dram_tensor("scales", [128, 2], mybir.dt.float32, kind="ExternalInput")
    # out block 0 stays zero (buffers are pre-zeroed); host keeps core 0's.
    out = nc.dram_tensor(
        "out", [E_PER_CORE + 1, 2 * ROW], mybir.dt.float16, kind="ExternalOutput"
    )

    bt_p128 = bt.rearrange("(n p) -> p n", p=128)  # [128, cols]: bt[n*128+p]

    with (
        ExitStack() as stack,
        nc.Block() as block,
    ):
        bufs = [
            stack.enter_context(
                nc.sbuf_tensor(f"buf{i}", [128, 2 * ROW], mybir.dt.float16)
            )
            for i in range(n_buf)
        ]
        btp32 = stack.enter_context(nc.sbuf_tensor("btp32", [128, cols], mybir.dt.int32))
        btpf = stack.enter_context(nc.sbuf_tensor("btpf", [128, cols], mybir.dt.float32))
        idx32 = stack.enter_context(nc.sbuf_tensor("idx32", [128, cols], mybir.dt.int32))
        valid = stack.enter_context(nc.sbuf_tensor("valid", [128, cols], mybir.dt.float32))
        ksv = stack.enter_context(nc.sbuf_tensor("ksv", [128, cols], mybir.dt.float32))
        vsv = stack.enter_context(nc.sbuf_tensor("vsv", [128, cols], mybir.dt.float32))
        scl = stack.enter_context(nc.sbuf_tensor("scl", [128, 2], mybir.dt.float32))

        load_sem = stack.enter_context(nc.semaphore("load"))
        vchain = stack.enter_context(nc.semaphore("vchain"))
        mulk_sem = stack.enter_context(nc.semaphore("mulk"))
        mulv_sem = stack.enter_context(nc.semaphore("mulv"))
        # Per-buffer DMA sems: concurrent DMAs on one shared sem would make
        # intermediate values ambiguous (increments from different DMAs mix).
        gather_sems = [
            stack.enter_context(nc.semaphore(f"gather{i}")) for i in range(n_buf)
        ]
        store_sems = [
            stack.enter_context(nc.semaphore(f"store{i}")) for i in range(n_buf)
        ]

        n_total = n_chunk * n_reps

        def chunk_aps(c):
            cc = c % n_chunk  # chunk within rep
            # chunk covers entries cc*chunk .. cc*chunk+chunk-1; with
            # chunk == 128 that is idx column cc, partitions 0..127.
            e0 = cc * chunk
            idx = idx32[:, cc : cc + 1]
            ks = ksv[:, cc : cc + 1]
            vs = vsv[:, cc : cc + 1]
            dst = out[1 + e0 : 1 + e0 + chunk, :]
            return idx, ks, vs, dst

        store_counts = [
            16 * len([c for c in range(n_total) if c % n_buf == b])
            for b in range(n_buf)
        ]

        def store_stream(eng, parity, n_engines):
            for c in range(n_total):
                if c % n_engines != parity:
                    continue
                _, _, _, dst = chunk_aps(c)
                eng.wait_ge(mulk_sem, c + 1)
                eng.wait_ge(mulv_sem, c + 1)
                eng.dma_start(dst, bufs[c % n_buf][:, :]).then_inc(
                    store_sems[c % n_buf], 16
                )

        @block.sync
        def _(sync: bass.BassEngine):
            # Prolog loads: bt partition-major and the scale pair.
            with nc.allow_non_contiguous_dma(reason="2KB one-time index load"):
                sync.dma_start(btp32[:, :], bt_p128[:, :]).then_inc(load_sem, 16)
            sync.dma_start(scl[:, :], scales[:, :]).then_inc(load_sem, 16)
            store_stream(sync, 0, 2 if split_store else 1)
            for b in range(n_buf):
                sync.wait_ge(store_sems[b], store_counts[b])

        @block.vector
        def _(vector: bass.BassVectorEngine):
            vector.wait_ge(load_sem, 16 * 2)
            # Block indices (clamped >= 0) and per-entry scales.
            # Same-engine RAW chains need explicit sync (deep pipeline).
            vector.tensor_copy(btpf[:, :], btp32[:, :]).then_inc(vchain, 1)
            vector.wait_ge(vchain, 1)
            vector.tensor_scalar_max(btpf[:, :], btpf[:, :], 0.0).then_inc(vchain, 1)
            vector.wait_ge(vchain, 2)
            vector.tensor_copy(idx32[:, :], btpf[:, :]).then_inc(vchain, 1)
            vector.tensor_scalar(
                valid[:, :], btpf[:, :], 0.0, None, op0=mybir.AluOpType.is_gt
            ).then_inc(vchain, 1)
            vector.wait_ge(vchain, 4)
            vector.tensor_scalar_mul(ksv[:, :], valid[:, :], scl[:, 0:1]).then_inc(
                vchain, 1
            )
            vector.tensor_scalar_mul(vsv[:, :], valid[:, :], scl[:, 1:2]).then_inc(
                vchain, 1
            )
            vector.wait_ge(vchain, 6)
            # Dequant loop: K halves.
            for c in range(n_total):
                _, ks, _, _ = chunk_aps(c)
                buf = bufs[c % n_buf]
                vector.wait_ge(gather_sems[c % n_buf], 16 * (c // n_buf + 1))
                vector.tensor_scalar_mul(
                    buf[:, 0:ROW], buf[:, 0:ROW], ks
                ).then_inc(mulk_sem, 1)

        @block.scalar
        def _(scalar: bass.BassScalarEngine):
            scalar.wait_ge(vchain, 6)
            # Dequant loop: V halves.
            for c in range(n_total):
                _, _, vs, _ = chunk_aps(c)
                buf = bufs[c % n_buf]
                scalar.wait_ge(gather_sems[c % n_buf], 16 * (c // n_buf + 1))
                scalar.mul(buf[:, ROW:], buf[:, ROW:], vs).then_inc(mulv_sem, 1)

        @block.gpsimd
        def _(gpsimd: bass.BassGpSimd):
            gpsimd.wait_ge(vchain, 3)  # idx32 written
            for c in range(n_total):
                idx, _, _, _ = chunk_aps(c)
                if c >= n_buf:
                    gpsimd.wait_ge(store_sems[c % n_buf], 16 * (c // n_buf))
                gpsimd.indirect_dma_start(
                    out=bufs[c % n_buf][:, :],
                    out_offset=None,
                    in_=kv[:, :],
                    in_offset=bass.IndirectOffsetOnAxis(ap=idx, axis=0),
                ).then_inc(gather_sems[c % n_buf], 16)

    nc.compile()
    return nc


def _get_nc() -> bass.Bass:
    global _NC_CACHE
    if _NC_CACHE is None:
        _NC_CACHE = build_nc()
    return _NC_CACHE


def _make_in_maps(inputs):
    kv = np.ascontiguousarray(np.asarray(inputs["kv_cache"])).view(np.float16)
    bt = np.asarray(inputs["block_tables"])
    k_scale = np.float32(inputs["k_scale"])
    v_scale = np.float32(inputs["v_scale"])

    kv_blocks = kv.reshape(NUM_BLOCKS, 2 * ROW)
    scales = np.empty((128, 2), np.float32)
    scales[:, 0] = k_scale
    scales[:, 1] = v_scale

    seq_per_core = BATCH // N_CORES
    in_maps = []
    for c in range(N_CORES):
        bt_shard = np.ascontiguousarray(
            bt[seq_per_core * c : seq_per_core * (c + 1)]
            .reshape(-1)
            .astype(np.int32)
        )
        in_maps.append({"kv": kv_blocks, "bt": bt_shard, "scales": scales})
    return in_maps


def _run(inputs, **kwargs) -> tuple[np.ndarray, "bass_utils.BassKernelResults"]:
    res = bass_utils.run_bass_kernel_spmd(
        _get_nc(), _make_in_maps(inputs), core_ids=list(range(N_CORES)), **kwargs
    )
    outs = [r["out"].reshape(E_PER_CORE + 1, 2, ROW) for r in res.results]
    full = np.empty((BATCH * MAX_BLOCKS_PER_SEQ + 1, 2, ROW), np.float16)
    full[0] = outs[0][0]
    for c in range(N_CORES):
        full[1 + E_PER_CORE * c : 1 + E_PER_CORE * (c + 1)] = outs[c][1:]
    return (
        full.reshape(-1, 2, NUM_KV_HEADS, BLOCK_SIZE, HEAD_DIM),
        res,
    )


def kernel(**inputs) -> np.ndarray:
    out, _ = _run(inputs)
    return out
